# revision 1
# baseline (speedup 1.0000x reference)
"""AreaAttention Trainium2 kernel (8 NeuronCores, batch-parallel).

Reference computation per batch element (C=64, H=W=256, AREA a=4):
  q = Wq@x+bq (8ch), k = Wk@x+bk (8ch), v = Wv@x+bv (64ch)   [1x1 convs]
  horizontal: per (row-block n, col w): A[i,j] = sum_c q[c,i]k[c,j],
     P = softmax_j(A);  out[c,j] = sum_i v[c,i] P[i,j]
  vertical: same over 4-col blocks per row.
  out = gamma*(h_out + v_out) + x

Per-core layout strategy:
  - x tile [64ch, pixels] in SBUF; one matmul with a column-replicated
    stationary produces q x4 replicas, k x4 replicas, v  -> qkv4 [128, pix].
  - scores: Q32 = diagonal-sliced q (row (i,c) = q[c] at i-offset) built with
    4 small SBUF->SBUF DMAs; 4 tensor_mul against the k-replica rows give the
    128-row pair-product tile; a 0/1 "ones" matmul reduces over c -> A[16,S].
  - softmax over j: exp on ACT, row-sum via second ones matmul (D broadcast
    over j built into the stationary), 1/D via exp(-log(D)) on ACT.
  - apply: per pair (i,j), broadcast P'[pair] to 64 partitions via a tiny
    matmul, then DVE multiply with v i-slice and accumulate into acc.
  - out = gamma*acc + x via one scalar_tensor_tensor.
"""

import sys
import numpy as np

for _p in ("/opt/trn_rl_repo", "/root/.axon_site/_ro/trn_rl_repo"):
    if _p not in sys.path:
        sys.path.insert(0, _p)

from contextlib import ExitStack

from concourse import bass, bacc, tile, mybir
from concourse.bass_utils import run_bass_kernel_spmd

F32 = mybir.dt.float32
AFT = mybir.ActivationFunctionType

# problem constants (hardcoded per the harness contract)
B, C, H, W = 8, 64, 256, 256
A_ = 4
C8 = 8
N_CORES = 8

# tiling
ROWS_T = 16                  # image rows per tile
N_TILES = H // ROWS_T        # 8
PT = ROWS_T * W              # 8192 pixels per tile
ST = PT // A_                # 2048 sites per direction per tile
CH = 512                     # sites per chunk
NCH = ST // CH               # 2
MM = 512                     # matmul moving-dim max (f32)

_cache = {}


def _consts(Wq, bq, Wk, bk, Wv, bv, gamma):
    W128 = np.zeros((C, 128), np.float32)
    b128 = np.zeros((128, 1), np.float32)
    for r in range(4):
        W128[:, r * 8:(r + 1) * 8] = Wq.T
        W128[:, 32 + r * 8:32 + (r + 1) * 8] = Wk.T
        b128[r * 8:(r + 1) * 8, 0] = bq
        b128[32 + r * 8:32 + (r + 1) * 8, 0] = bk
    W128[:, 64:] = Wv.T
    b128[64:, 0] = bv

    # prod row p = 32*j + 8*i + c  ->  pair col i*4+j
    Osel = np.zeros((128, 16), np.float32)
    for p in range(128):
        j, i, c = p >> 5, (p >> 3) & 3, p & 7
        Osel[p, i * 4 + j] = 1.0

    # D16 row (i',j') = sum_j E[(i',j)]
    Osum = np.zeros((16, 16), np.float32)
    for p in range(16):
        for p2 in range(16):
            if p >> 2 == p2 >> 2:
                Osum[p, p2] = 1.0

    # per-pair broadcast to 64 partitions
    Odiag = np.zeros((16, 16 * 64), np.float32)
    for pair in range(16):
        Odiag[pair, pair * 64:(pair + 1) * 64] = 1.0

    g64 = np.full((64, 1), np.float32(gamma[0]), np.float32)
    return W128, b128, Osel, Osum, Odiag, g64


def _build():
    nc = bacc.Bacc("TRN2", target_bir_lowering=False, debug=False,
                   num_devices=N_CORES)
    x_d = nc.dram_tensor("x", [C, H, W], F32, kind="ExternalInput")
    W128_d = nc.dram_tensor("W128", [C, 128], F32, kind="ExternalInput")
    b128_d = nc.dram_tensor("b128", [128, 1], F32, kind="ExternalInput")
    Osel_d = nc.dram_tensor("Osel", [128, 16], F32, kind="ExternalInput")
    Osum_d = nc.dram_tensor("Osum", [16, 16], F32, kind="ExternalInput")
    Odiag_d = nc.dram_tensor("Odiag", [16, 16 * 64], F32, kind="ExternalInput")
    g64_d = nc.dram_tensor("g64", [64, 1], F32, kind="ExternalInput")
    out_d = nc.dram_tensor("out", [C, H, W], F32, kind="ExternalOutput")

    with tile.TileContext(nc) as tc, ExitStack() as ctx:
        consts = ctx.enter_context(tc.tile_pool(name="consts", bufs=1))
        xp = ctx.enter_context(tc.tile_pool(name="xp", bufs=2))
        qkvp = ctx.enter_context(tc.tile_pool(name="qkvp", bufs=2))
        accp = ctx.enter_context(tc.tile_pool(name="accp", bufs=2))
        outp = ctx.enter_context(tc.tile_pool(name="outp", bufs=2))
        small = ctx.enter_context(tc.tile_pool(name="small", bufs=2))
        tmpp = ctx.enter_context(tc.tile_pool(name="tmpp", bufs=3))
        q32p = ctx.enter_context(tc.tile_pool(name="q32p", bufs=2))
        prodp = ctx.enter_context(tc.tile_pool(name="prodp", bufs=2))
        ps_qkv = ctx.enter_context(
            tc.tile_pool(name="ps_qkv", bufs=2, space="PSUM"))
        ps_a = ctx.enter_context(tc.tile_pool(name="ps_a", bufs=1, space="PSUM"))
        ps_d = ctx.enter_context(tc.tile_pool(name="ps_d", bufs=1, space="PSUM"))
        ps_pb = ctx.enter_context(tc.tile_pool(name="ps_pb", bufs=1, space="PSUM"))

        W128_s = consts.tile([C, 128], F32)
        b128_s = consts.tile([128, 1], F32)
        Osel_s = consts.tile([128, 16], F32)
        Osum_s = consts.tile([16, 16], F32)
        Odiag_s = consts.tile([16, 16 * 64], F32)
        g64_s = consts.tile([64, 1], F32)
        for t, d in ((W128_s, W128_d), (b128_s, b128_d), (Osel_s, Osel_d),
                     (Osum_s, Osum_d), (Odiag_s, Odiag_d), (g64_s, g64_d)):
            nc.sync.dma_start(t[:], d[:])

        for it in range(N_TILES):
            r0 = it * ROWS_T
            x_t = xp.tile([C, PT], F32, tag="x")
            nc.sync.dma_start(x_t[:], x_d[:, r0:r0 + ROWS_T, :])

            # ---- qkv (+replicas) ----
            qkv = qkvp.tile([128, PT], F32, tag="qkv")
            for m in range(PT // MM):
                ps = ps_qkv.tile([128, MM], F32, tag="psqkv")
                nc.tensor.matmul(ps[:], W128_s[:], x_t[:, m * MM:(m + 1) * MM],
                                 start=True, stop=True)
                nc.scalar.activation(qkv[:, m * MM:(m + 1) * MM], ps[:],
                                     AFT.Identity, bias=b128_s[:])

            acc = accp.tile([64, PT], F32, tag="acc")

            for d in range(2):  # 0=horizontal, 1=vertical
                if d == 0:
                    # sites (n, w): pixel (n, i, w); free view [8, 4, 256]
                    qv = qkv[:, :].rearrange("p (n i w) -> p n i w", i=A_, w=W)
                    av = acc[:, :].rearrange("p (n i w) -> p n i w", i=A_, w=W)
                else:
                    # sites (h, nb): pixel (h, nb, j); free view [32, 64, 4]
                    qv = qkv[:, :].rearrange("p (h nb j) -> p h nb j", nb=W // A_, j=A_)
                    av = acc[:, :].rearrange("p (h nb j) -> p h nb j", nb=W // A_, j=A_)

                # Q32: row (i,c) = q[c] at slice i  (4 small sb2sb DMAs).
                # Data parked at rows 32:64 so the tensor_mul below sees the
                # same base partition as the k-replica rows (HW requirement).
                q32 = q32p.tile([64, ST], F32, tag="q32")
                for i in range(A_):
                    if d == 0:
                        src = qv[i * 8:(i + 1) * 8, :, i, :]
                    else:
                        src = qv[i * 8:(i + 1) * 8, :, :, i]
                    nc.sync.dma_start(q32[32 + i * 8:32 + (i + 1) * 8, :], src)

                # prod[32j+8i+c] = q[c]@i  *  k[c]@j
                prod = prodp.tile([128, ST], F32, tag="prod")
                for j in range(A_):
                    kj = (qv[32:64, :, j, :] if d == 0 else qv[32:64, :, :, j])
                    nc.vector.tensor_mul(prod[j * 32:(j + 1) * 32, :],
                                         q32[32:64, :], kj)

                for chk in range(NCH):
                    s0 = chk * CH
                    a_ps = ps_a.tile([16, CH], F32, tag="aps")
                    d_ps = ps_d.tile([16, CH], F32, tag="dps")
                    for h in range(CH // MM):
                        nc.tensor.matmul(
                            a_ps[:, h * MM:(h + 1) * MM], Osel_s[:],
                            prod[:, s0 + h * MM:s0 + (h + 1) * MM],
                            start=True, stop=True)
                    E = small.tile([16, CH], F32, tag="E")
                    nc.scalar.activation(E[:], a_ps[:], AFT.Exp)
                    for h in range(CH // MM):
                        nc.tensor.matmul(d_ps[:, h * MM:(h + 1) * MM], Osum_s[:],
                                         E[:, h * MM:(h + 1) * MM],
                                         start=True, stop=True)
                    Lg = small.tile([16, CH], F32, tag="Lg")
                    nc.scalar.activation(Lg[:], d_ps[:], AFT.Ln)
                    R = small.tile([16, CH], F32, tag="R")
                    nc.scalar.activation(R[:], Lg[:], AFT.Exp, scale=-1.0)
                    Pp = small.tile([16, CH], F32, tag="Pp")
                    nc.vector.tensor_mul(Pp[:], E[:], R[:])

                    for pair in range(16):
                        i, j = pair >> 2, pair & 3
                        pb = ps_pb.tile([64, CH], F32, tag="pb")
                        for h in range(CH // MM):
                            nc.tensor.matmul(
                                pb[:, h * MM:(h + 1) * MM],
                                Odiag_s[:, pair * 64:(pair + 1) * 64],
                                Pp[:, h * MM:(h + 1) * MM],
                                start=True, stop=True)
                        if d == 0:
                            nrng = slice(chk * 2, (chk + 1) * 2)
                            vi = qv[64:, nrng, i, :]
                            aj = av[:, nrng, j, :]
                        else:
                            hrng = slice(chk * 8, (chk + 1) * 8)
                            vi = qv[64:, hrng, :, i]
                            aj = av[:, hrng, :, j]
                        pbv = pb[:, :].rearrange(
                            "p (a b) -> p a b", a=(2 if d == 0 else 8))
                        if d == 0 and i == 0:
                            nc.vector.tensor_mul(aj, pbv, vi)
                        else:
                            tmp = tmpp.tile([64, CH], F32, tag="tmp")
                            tv = tmp[:, :].rearrange(
                                "p (a b) -> p a b", a=(2 if d == 0 else 8))
                            nc.vector.tensor_mul(tv, pbv, vi)
                            nc.vector.tensor_add(aj, aj, tv)

            # out = gamma*acc + x
            o_t = outp.tile([64, PT], F32, tag="o")
            nc.vector.scalar_tensor_tensor(
                o_t[:], acc[:], g64_s[:], x_t[:],
                op0=mybir.AluOpType.mult, op1=mybir.AluOpType.add)
            nc.sync.dma_start(out_d[:, r0:r0 + ROWS_T, :], o_t[:])

    nc.compile()
    return nc


def _run(x, Wq, bq, Wk, bk, Wv, bv, gamma, **spmd_kwargs):
    x = np.asarray(x, np.float32)
    W128, b128, Osel, Osum, Odiag, g64 = _consts(
        np.asarray(Wq, np.float32), np.asarray(bq, np.float32),
        np.asarray(Wk, np.float32), np.asarray(bk, np.float32),
        np.asarray(Wv, np.float32), np.asarray(bv, np.float32),
        np.asarray(gamma, np.float32))

    if "nc" not in _cache:
        _cache["nc"] = _build()
    nc = _cache["nc"]

    in_maps = []
    for b in range(N_CORES):
        in_maps.append({"x": np.ascontiguousarray(x[b]), "W128": W128,
                        "b128": b128, "Osel": Osel, "Osum": Osum,
                        "Odiag": Odiag, "g64": g64})
    res = run_bass_kernel_spmd(nc, in_maps, core_ids=list(range(N_CORES)),
                               **spmd_kwargs)
    out = np.stack([res.results[b]["out"] for b in range(N_CORES)], axis=0)
    return out, res


def kernel(x, Wq, bq, Wk, bk, Wv, bv, gamma):
    return _run(x, Wq, bq, Wk, bk, Wv, bv, gamma)[0]

